# revision 1
# baseline (speedup 1.0000x reference)
"""KGram MLP seq model (embedding_lookup) on 8 Trainium2 NeuronCores.

Computation: emb[s,b] = sum_j W1t[token(s,b,j) + j*V] + b1 ; h = SiLU(emb)
             logits = h @ W2 + b2                      -> (2048, 2, 32000) f32

Sharding: vocab-parallel. Each core holds W2 columns [c*4000, (c+1)*4000),
computes the full h (the 3-row gathers are cheap and hidden under the
matmul), and produces its (4096, 4000) logits slice; the host concatenates.

Device pipeline per core (all positions flattened to 4096 = 2048*2 rows):
  - W1t rows are deduplicated on host to a compact bf16 table (<= 3*4098 rows)
    and gathered on-device with dma_gather(transpose=True), which lands the
    embeddings directly in [embed%128 (partition), embed//128, position]
    layout -- exactly what the matmul needs as the stationary operand.
  - 3-row sum + SiLU(+b1) in bf16 via DVE/ACT.
  - h @ W2 in bf16 (fp32 PSUM accumulate), 128x512 tiles, 8 PSUM banks.
  - +b2 during PSUM->SBUF eviction (DVE), f32 out, DMA to DRAM.
"""

import numpy as np
import ml_dtypes

VOCAB = 32000
KGRAM = 3
EMBED = 1024
SEQ = 2048
BATCH = 2
POS = SEQ * BATCH            # 4096 flattened positions (s-major, batch minor)
NCORES = 8
VSH = VOCAB // NCORES        # 4000 vocab columns per core
UPAD = 4098                  # padded unique-token count (4096 tokens + pad 0 + slack)
RTAB = KGRAM * UPAD          # compact table rows (12294 < int16 max)
P = 128
CB = EMBED // P              # 8 embed column blocks
KT = EMBED // P              # 8 contraction tiles
CHUNK = 256                  # positions per gather chunk
NCH = POS // CHUNK           # 16
GIDX = KGRAM * CHUNK         # 768 gather indices per chunk
IDXC = GIDX // 16            # 48 idx columns per chunk
NT = [512] * 7 + [416]       # vocab n-tile sizes (sum 4000)
NOFF = [0, 512, 1024, 1536, 2048, 2560, 3072, 3584]

_BF16 = ml_dtypes.bfloat16

_CACHED_NC = None


def _build_nc():
    """Build + compile the per-core Bass program (identical on all 8 cores)."""
    from contextlib import ExitStack

    import concourse.bacc as bacc
    import concourse.tile as tile
    import concourse.mybir as mybir

    dtbf = mybir.dt.bfloat16
    dtf = mybir.dt.float32
    dti = mybir.dt.int16

    nc = bacc.Bacc("TRN2", target_bir_lowering=False, debug=False,
                   num_devices=NCORES)

    table = nc.dram_tensor("table", [RTAB, EMBED], dtbf, kind="ExternalInput")
    idx = nc.dram_tensor("idx", [P, NCH * IDXC], dti, kind="ExternalInput")
    b1t = nc.dram_tensor("b1t", [P, CB], dtf, kind="ExternalInput")
    w2b = nc.dram_tensor("w2b", [P, KT * VSH], dtbf, kind="ExternalInput")
    b2r = nc.dram_tensor("b2r", [P, VSH], dtf, kind="ExternalInput")
    out = nc.dram_tensor("out", [POS, VSH], dtf, kind="ExternalOutput")

    ident = mybir.ActivationFunctionType.Identity
    sigm = mybir.ActivationFunctionType.Sigmoid

    with tile.TileContext(nc) as tc, ExitStack() as ctx:
        const = ctx.enter_context(tc.tile_pool(name="const", bufs=1))
        w2s = const.tile([P, KT * VSH], dtbf, tag="w2s")
        nc.sync.dma_start(w2s[:], w2b.ap())
        idxs = const.tile([P, NCH * IDXC], dti, tag="idxs")
        nc.sync.dma_start(idxs[:], idx.ap())
        b1s = const.tile([P, CB], dtf, tag="b1s")
        nc.sync.dma_start(b1s[:], b1t.ap())
        b2s = const.tile([P, VSH], dtf, tag="b2s")
        nc.sync.dma_start(b2s[:], b2r.ap())

        gpool = ctx.enter_context(tc.tile_pool(name="g", bufs=2))
        spool = ctx.enter_context(tc.tile_pool(name="s", bufs=2))
        hpool = ctx.enter_context(tc.tile_pool(name="h", bufs=2))
        opool = ctx.enter_context(tc.tile_pool(name="o", bufs=2))
        psum = ctx.enter_context(tc.tile_pool(name="ps", bufs=8, space="PSUM"))

        for c in range(NCH):
            g = gpool.tile([P, CB, GIDX], dtbf, tag="g")
            nc.gpsimd.dma_gather(
                g[:], table.ap(), idxs[:, c * IDXC:(c + 1) * IDXC],
                GIDX, GIDX, EMBED, transpose=True,
            )
            s1 = spool.tile([P, CB, CHUNK], dtbf, tag="s1")
            nc.vector.tensor_add(s1[:], g[:, :, 0:CHUNK], g[:, :, CHUNK:2 * CHUNK])
            s2 = spool.tile([P, CB, CHUNK], dtbf, tag="s2")
            nc.vector.tensor_add(s2[:], s1[:], g[:, :, 2 * CHUNK:3 * CHUNK])
            # SiLU(s2 + b1) = e * sigmoid(e), e = s2 + b1 (per-partition bias)
            e = spool.tile([P, CB, CHUNK], dtbf, tag="e")
            sg = spool.tile([P, CB, CHUNK], dtbf, tag="sg")
            for cb in range(CB):
                nc.scalar.activation(e[:, cb, :], s2[:, cb, :], ident,
                                     bias=b1s[:, cb:cb + 1], scale=1.0)
                nc.scalar.activation(sg[:, cb, :], s2[:, cb, :], sigm,
                                     bias=b1s[:, cb:cb + 1], scale=1.0)
            h = hpool.tile([P, CB, CHUNK], dtbf, tag="h")
            nc.vector.tensor_mul(h[:], e[:], sg[:])

            for m in range(CHUNK // P):
                o = opool.tile([P, VSH], dtf, tag="o")
                pts = [psum.tile([P, 512], dtf, tag="ps", name=f"ps{n}")
                       for n in range(8)]
                for k in range(KT):
                    lhsT = h[:, k, m * P:(m + 1) * P]
                    for n in range(8):
                        nc.tensor.matmul(
                            pts[n][:, :NT[n]], lhsT,
                            w2s[:, k * VSH + NOFF[n]: k * VSH + NOFF[n] + NT[n]],
                            start=(k == 0), stop=(k == KT - 1),
                        )
                for n in range(8):
                    nc.vector.tensor_add(o[:, NOFF[n]:NOFF[n] + NT[n]],
                                         pts[n][:, :NT[n]],
                                         b2s[:, NOFF[n]:NOFF[n] + NT[n]])
                r0 = c * CHUNK + m * P
                nc.sync.dma_start(out.ap()[r0:r0 + P, :], o[:])

    nc.compile()
    return nc


def get_nc():
    global _CACHED_NC
    if _CACHED_NC is None:
        _CACHED_NC = _build_nc()
    return _CACHED_NC


def _prep_inputs(tokens_seq, W1t, b1, W2, b2):
    """Host-side sharding/layout. Returns in_maps for the 8 cores."""
    tokens = np.asarray(tokens_seq).astype(np.int64)
    assert tokens.shape == (SEQ, BATCH)
    W1t = np.asarray(W1t, dtype=np.float32)
    b1 = np.asarray(b1, dtype=np.float32)
    W2 = np.asarray(W2, dtype=np.float32)
    b2 = np.asarray(b2, dtype=np.float32)

    padded = np.concatenate(
        [np.zeros((KGRAM - 1, BATCH), dtype=np.int64), tokens], axis=0)
    uniq, inv = np.unique(padded, return_inverse=True)
    inv = inv.reshape(padded.shape)
    U = len(uniq)
    assert U <= UPAD

    # compact bf16 table: row j*UPAD + u  <-  W1t[j*VOCAB + uniq[u]]
    table = np.zeros((RTAB, EMBED), dtype=_BF16)
    for j in range(KGRAM):
        table[j * UPAD:j * UPAD + U] = W1t[j * VOCAB + uniq].astype(_BF16)

    # gather index stream per chunk: [j=0 positions..., j=1 ..., j=2 ...]
    # cid(j, pos) = j*UPAD + inv[s+j, b] with pos = s*BATCH + b
    cidx = np.empty((KGRAM, POS), dtype=np.int16)
    for j in range(KGRAM):
        cidx[j] = (j * UPAD + inv[j:j + SEQ, :]).reshape(-1).astype(np.int16)
    idx_host = np.empty((P, NCH * IDXC), dtype=np.int16)
    for c in range(NCH):
        stream = np.concatenate(
            [cidx[j, c * CHUNK:(c + 1) * CHUNK] for j in range(KGRAM)])
        blk = stream.reshape(IDXC, 16).T      # [i%16, i//16]
        idx_host[:, c * IDXC:(c + 1) * IDXC] = np.tile(blk, (8, 1))

    b1t = np.ascontiguousarray(b1.reshape(CB, P).T)          # [p, cb] = b1[cb*128+p]

    w2r = W2.reshape(KT, P, VOCAB)
    in_maps = []
    for core in range(NCORES):
        v0 = core * VSH
        w2b = np.ascontiguousarray(
            w2r[:, :, v0:v0 + VSH].transpose(1, 0, 2)).reshape(P, KT * VSH)
        in_maps.append({
            "table": table,
            "idx": idx_host,
            "b1t": b1t,
            "w2b": w2b.astype(_BF16),
            "b2r": np.ascontiguousarray(
                np.broadcast_to(b2[v0:v0 + VSH], (P, VSH))),
        })
    return in_maps


def run(tokens_seq, W1t, b1, W2, b2, trace=False):
    """Run on 8 cores; returns (logits, BassKernelResults)."""
    from concourse.bass_utils import run_bass_kernel_spmd

    nc = get_nc()
    in_maps = _prep_inputs(tokens_seq, W1t, b1, W2, b2)
    res = run_bass_kernel_spmd(nc, in_maps, list(range(NCORES)), trace=trace)
    parts = [res.results[i]["out"] for i in range(NCORES)]
    logits = np.concatenate(parts, axis=1).reshape(SEQ, BATCH, VOCAB)
    return logits, res


def kernel(tokens_seq, W1t, b1, W2, b2):
    logits, _ = run(tokens_seq, W1t, b1, W2, b2)
    return logits



# revision 3
# speedup vs baseline: 11.1921x; 11.1921x over previous
"""KGram MLP seq model (embedding_lookup) on 8 Trainium2 NeuronCores.

Computation: emb[s,b] = sum_j W1t[token(s,b,j) + j*V] + b1 ; h = SiLU(emb)
             logits = h @ W2 + b2                      -> (2048, 2, 32000) f32

Sharding: vocab-parallel. Each core holds a (1024, 4000) W2 column slice,
computes the full h redundantly (cheap, hidden under the matmul), and
produces its (4096, 4000) logits slice; the host concatenates. No
collectives.

Device pipeline per core, per chunk of 256 positions (16 chunks):
 - W1t rows are deduplicated on host into a compact bf16 table and gathered
   on-device with dma_gather(transpose=True), landing embeddings directly in
   [embed%128 (partition), embed//128, position] layout.
 - DVE: 3-way k-gram sum (bf16); ACT: h = Silu(sum + b1) in one pass.
 - DVE: h8 = fp8e4(h * 64)  (quantize for the fast matmul path).
 - PE fp8 DoubleRow matmul: each instruction contracts TWO 128-deep k-tiles
   (stationary h8 as [Ki=128, Ko=2, M=128], moving W2 as fp8 half-planes
   [128, 2, N<=512], W2 pre-scaled x512 on host) accumulating f32 in PSUM.
   1024 MMs instead of 2048; ~1.7x faster than the bf16 stream on HW.
   PSUM is split into two ping-ponged 4-bank waves per 128-position block so
   evictions never stall the next accumulation group.
 - ACT evicts each bank with the 2^-15 rescale (frees PSUM early); DVE adds
   b2; DMA writes the f32 logits slice.
Measured steady-state device time: ~290 us per full computation (vs ~590 us
for the bf16 baseline); absmax relative error ~6.3e-3 (gate: 2e-2).
"""
import numpy as np
import ml_dtypes

VOCAB = 32000
KGRAM = 3
EMBED = 1024
SEQ = 2048
BATCH = 2
POS = SEQ * BATCH
NCORES = 8
VSH = VOCAB // NCORES
UPAD = 4104
RTAB = KGRAM * UPAD
P = 128
CB = EMBED // P
KT = EMBED // P
KTS = KT // 2                # 4 double-row super k-tiles
CHUNK = 256
NCH = POS // CHUNK
GIDX = KGRAM * CHUNK
IDXC = GIDX // 16
NTS = [[512, 512, 512, 512], [512, 512, 512, 416]]
WOFF = [0, 2048]

H_SCALE = 64.0
W_SCALE = 512.0
OUT_SCALE = 1.0 / (H_SCALE * W_SCALE)     # 2^-15

_BF16 = ml_dtypes.bfloat16
_FP8 = ml_dtypes.float8_e4m3


def build_nc(reps=1):
    from contextlib import ExitStack

    import concourse.bacc as bacc
    import concourse.tile as tile
    import concourse.mybir as mybir

    dtbf = mybir.dt.bfloat16
    dtf = mybir.dt.float32
    dti = mybir.dt.int16
    dt8 = mybir.dt.float8e4

    nc = bacc.Bacc("TRN2", target_bir_lowering=False, debug=False,
                   num_devices=NCORES)

    table = nc.dram_tensor("table", [RTAB, EMBED], dtbf, kind="ExternalInput")
    idx = nc.dram_tensor("idx", [P, NCH * IDXC], dti, kind="ExternalInput")
    b1t = nc.dram_tensor("b1t", [P, CB], dtf, kind="ExternalInput")
    w2b = nc.dram_tensor("w2b", [P, KTS, 2, VSH], dt8, kind="ExternalInput")
    b2r = nc.dram_tensor("b2r", [P, VSH], dtf, kind="ExternalInput")
    out = nc.dram_tensor("out", [POS, VSH], dtf, kind="ExternalOutput")

    silu = mybir.ActivationFunctionType.Silu
    ident = mybir.ActivationFunctionType.Identity
    drow = mybir.MatmulPerfMode.DoubleRow

    with tile.TileContext(nc) as tc, ExitStack() as ctx:
        const = ctx.enter_context(tc.tile_pool(name="const", bufs=1))
        w2s = const.tile([P, KTS, 2, VSH], dt8, tag="w2s")
        nc.sync.dma_start(w2s[:], w2b.ap())
        idxs = const.tile([P, NCH * IDXC], dti, tag="idxs")
        nc.sync.dma_start(idxs[:], idx.ap())
        b1s = const.tile([P, CB], dtf, tag="b1s")
        nc.sync.dma_start(b1s[:], b1t.ap())
        b2s = const.tile([P, VSH], dtf, tag="b2s")
        nc.sync.dma_start(b2s[:], b2r.ap())

        gpool = ctx.enter_context(tc.tile_pool(name="g", bufs=2))
        spool = ctx.enter_context(tc.tile_pool(name="s", bufs=2))
        hpool = ctx.enter_context(tc.tile_pool(name="h", bufs=2))
        opool = ctx.enter_context(tc.tile_pool(name="o", bufs=2))
        psum = ctx.enter_context(tc.tile_pool(name="ps", bufs=2, space="PSUM"))

        for r in range(reps):
            for c in range(NCH):
                g = gpool.tile([P, CB, GIDX], dtbf, tag="g")
                nc.gpsimd.dma_gather(
                    g[:], table.ap(), idxs[:, c * IDXC:(c + 1) * IDXC],
                    GIDX, GIDX, EMBED, transpose=True,
                )
                s1 = spool.tile([P, CB, CHUNK], dtbf, tag="s1")
                nc.vector.tensor_add(s1[:], g[:, :, 0:CHUNK], g[:, :, CHUNK:2 * CHUNK])
                s2 = spool.tile([P, CB, CHUNK], dtbf, tag="s2")
                nc.vector.tensor_add(s2[:], s1[:], g[:, :, 2 * CHUNK:3 * CHUNK])
                h = hpool.tile([P, CB, CHUNK], dtbf, tag="h")
                for cb in range(CB):
                    nc.scalar.activation(h[:, cb, :], s2[:, cb, :], silu,
                                         bias=b1s[:, cb:cb + 1], scale=1.0)
                h8 = hpool.tile([P, CB, CHUNK], dt8, tag="h8")
                nc.vector.tensor_scalar_mul(h8[:], h[:], H_SCALE)

                for m in range(CHUNK // P):
                    o = opool.tile([P, VSH], dtf, tag="o")
                    for w, nts in enumerate(NTS):
                        pts = psum.tile([P, 4, 512], dtf, tag="ps")
                        for kt2 in range(KTS):
                            lhsT = h8[:, 2 * kt2:2 * kt2 + 2, m * P:(m + 1) * P]
                            for j, nt in enumerate(nts):
                                noff = WOFF[w] + j * 512
                                nc.tensor.matmul(
                                    pts[:, j, :nt], lhsT,
                                    w2s[:, kt2, :, noff:noff + nt],
                                    start=(kt2 == 0), stop=(kt2 == KTS - 1),
                                    perf_mode=drow,
                                )
                        for j, nt in enumerate(nts):
                            noff = WOFF[w] + j * 512
                            nc.scalar.activation(o[:, noff:noff + nt],
                                                 pts[:, j, :nt], ident,
                                                 scale=OUT_SCALE)
                        for j, nt in enumerate(nts):
                            noff = WOFF[w] + j * 512
                            nc.vector.tensor_add(o[:, noff:noff + nt],
                                                 o[:, noff:noff + nt],
                                                 b2s[:, noff:noff + nt])
                    r0 = c * CHUNK + m * P
                    nc.sync.dma_start(out.ap()[r0:r0 + P, :], o[:])

    nc.compile()
    return nc


_CACHED = {}


def get_nc(reps=1):
    if reps not in _CACHED:
        _CACHED[reps] = build_nc(reps)
    return _CACHED[reps]


def _prep_inputs(tokens_seq, W1t, b1, W2, b2):
    tokens = np.asarray(tokens_seq).astype(np.int64)
    assert tokens.shape == (SEQ, BATCH)
    W1t = np.asarray(W1t, dtype=np.float32)
    b1 = np.asarray(b1, dtype=np.float32)
    W2 = np.asarray(W2, dtype=np.float32)
    b2 = np.asarray(b2, dtype=np.float32)

    padded = np.concatenate(
        [np.zeros((KGRAM - 1, BATCH), dtype=np.int64), tokens], axis=0)
    uniq, inv = np.unique(padded, return_inverse=True)
    inv = inv.reshape(padded.shape)
    U = len(uniq)
    assert U <= UPAD

    table = np.zeros((RTAB, EMBED), dtype=_BF16)
    for j in range(KGRAM):
        table[j * UPAD:j * UPAD + U] = W1t[j * VOCAB + uniq].astype(_BF16)

    cidx = np.empty((KGRAM, POS), dtype=np.int16)
    for j in range(KGRAM):
        cidx[j] = (j * UPAD + inv[j:j + SEQ, :]).reshape(-1).astype(np.int16)
    idx_host = np.empty((P, NCH * IDXC), dtype=np.int16)
    for c in range(NCH):
        stream = np.concatenate(
            [cidx[j, c * CHUNK:(c + 1) * CHUNK] for j in range(KGRAM)])
        blk = stream.reshape(IDXC, 16).T
        idx_host[:, c * IDXC:(c + 1) * IDXC] = np.tile(blk, (8, 1))

    b1t = np.ascontiguousarray(b1.reshape(CB, P).T)

    # fp8 half-plane W2: w2b[p, kt2, ko, n] = fp8(W2[(2kt2+ko)*128+p, n] * 512)
    w28 = (W2 * W_SCALE).astype(_FP8)          # [1024, 32000]
    w28 = w28.reshape(KTS, 2, P, VOCAB)        # [kt2, ko, p, v]
    in_maps = []
    for core in range(NCORES):
        v0 = core * VSH
        sl = w28[:, :, :, v0:v0 + VSH]         # [kt2, ko, p, n]
        w2b = np.ascontiguousarray(sl.transpose(2, 0, 1, 3))
        in_maps.append({
            "table": table,
            "idx": idx_host,
            "b1t": b1t,
            "w2b": w2b,
            "b2r": np.ascontiguousarray(
                np.broadcast_to(b2[v0:v0 + VSH], (P, VSH))),
        })
    return in_maps


def run(tokens_seq, W1t, b1, W2, b2):
    from concourse.bass_utils import run_bass_kernel_spmd

    nc = get_nc(1)
    in_maps = _prep_inputs(tokens_seq, W1t, b1, W2, b2)
    res = run_bass_kernel_spmd(nc, in_maps, list(range(NCORES)))
    parts = [res.results[i]["out"] for i in range(NCORES)]
    logits = np.concatenate(parts, axis=1).reshape(SEQ, BATCH, VOCAB)
    return logits, res


def kernel(tokens_seq, W1t, b1, W2, b2):
    logits, _ = run(tokens_seq, W1t, b1, W2, b2)
    return logits
